# revision 1
# baseline (speedup 1.0000x reference)
"""Bi-directional WKV (RWKV-style) kernel for Trainium2, 8-core batch-parallel.

Math per (b, t, c):
    tf_b     = sigmoid(sum(time_emb[b]))
    decay_bc = exp(-exp(w_c)) * (0.5 + 0.5*tf_b)
    ek   = exp(k);  ekv = ek * v
    nf_t = decay*nf_{t-1} + ekv_t    (forward inclusive scan over T)
    df_t = decay*df_{t-1} + ek_t
    nb_t = decay*nb_{t+1} + ekv_t    (backward inclusive scan)
    db_t = decay*db_{t+1} + ek_t
    num  = nf + nb - ekv + exp(u+k)*v = nf + nb + (e^u - 1)*ekv
    den  = df + db - ek  + exp(u+k)   = df + db + (e^u - 1)*ek
    out  = sigmoid(r) * num/den * (0.8 + 0.2*tf_b)

Layout strategy per core (one batch element, [T=2048, C=2048] f32):
  - process per ctile of 128 channels; SBUF tiles are [128 chan, T] so the
    T-recurrence runs along the free dim.
  - DRAM->SBUF loads are 1MB "slabs" [128 tpart, 16 tchunk, 128 chan]
    (512B contiguous runs); PE transposes 128x128 blocks into [chan, T].
  - ekv/ek are interleaved along the free dim so one scalar_tensor_tensor
    and one add serve both the num and den assemblies.
  - division and the sigmoid factor run in log domain on the Scalar
    engine: out = num * exp(-(ln(den) + ln(1+e^-r)) + ln(scale_b)).
"""

import numpy as np
from contextlib import ExitStack

import concourse.bass as bass
import concourse.bacc as bacc
import concourse.tile as tile
from concourse import mybir
from concourse.bass_utils import run_bass_kernel_spmd
from concourse.masks import make_identity


# Confine ACT table-set choice: every Exp/Ln/Copy/Identity instruction can be
# served by 'natural_log_exp_and_others'; leaving the other sets visible makes
# the table-load pass bounce between 'small'/'natural_log'/'exp_and_friends'
# (~1.3us per reload, several per ctile). Neutering the alternatives (in the
# cached dict the pass reads) pins one set for the whole kernel.
from concourse.hw_specs import get_activation_tables


def _pin_act_tables():
    tabs = get_activation_tables("gen3")
    keep = {"natural_log_exp_and_others", "sigmoid_and_friends"}
    for name in list(tabs):
        if name not in keep:
            tabs[name] = set()


_pin_act_tables()

B, T, C, TD = 8, 2048, 2048, 512
P = 128
N_CORES = 8
f32 = mybir.dt.float32
ALU = mybir.AluOpType
AF = mybir.ActivationFunctionType


def _body(tc, out, r, k, v, w, u, te, T_, C_, TD_):
    nc = tc.nc
    NT = T_ // P   # number of t-chunks
    NCT = C_ // P  # number of c-tiles

    with ExitStack() as ctx:
        consts = ctx.enter_context(tc.tile_pool(name="consts", bufs=1))
        slabs = ctx.enter_context(tc.tile_pool(name="slabs", bufs=2))
        cbuf = ctx.enter_context(tc.tile_pool(name="cbuf", bufs=2))
        scano = ctx.enter_context(tc.tile_pool(name="scano", bufs=1))
        psum = ctx.enter_context(tc.tile_pool(name="psum", bufs=2, space="PSUM"))

        ident = consts.tile([P, P], f32)
        make_identity(nc, ident[:])

        # ---- per-batch time factor, on all 128 partitions ----
        te_t = consts.tile([P, TD_], f32)
        te_b = bass.AP(tensor=te.tensor, offset=te.offset, ap=[[0, P]] + list(te.ap))
        nc.gpsimd.dma_start(out=te_t[:], in_=te_b)
        ssum = consts.tile([P, 1], f32)
        nc.vector.tensor_reduce(out=ssum[:], in_=te_t[:], axis=mybir.AxisListType.X,
                                op=ALU.add)
        tf = consts.tile([P, 1], f32)
        nc.scalar.activation(out=tf[:], in_=ssum[:], func=AF.Sigmoid)
        scale_b = consts.tile([P, 1], f32)   # 0.8 + 0.2*tf
        nc.vector.tensor_scalar(out=scale_b[:], in0=tf[:], scalar1=0.2, scalar2=0.8,
                                op0=ALU.mult, op1=ALU.add)
        htf = consts.tile([P, 1], f32)       # 0.5 + 0.5*tf
        nc.vector.tensor_scalar(out=htf[:], in0=tf[:], scalar1=0.5, scalar2=0.5,
                                op0=ALU.mult, op1=ALU.add)

        # ---- per-channel constants, [128, NCT] (partition = chan within tile) ----
        wt = consts.tile([P, NCT], f32)
        nc.gpsimd.dma_start(out=wt[:], in_=w.rearrange("(j p) -> p j", p=P))
        ut = consts.tile([P, NCT], f32)
        nc.gpsimd.dma_start(out=ut[:], in_=u.rearrange("(j p) -> p j", p=P))
        ew = consts.tile([P, NCT], f32)
        nc.scalar.activation(out=ew[:], in_=wt[:], func=AF.Exp)          # e^w
        dec0 = consts.tile([P, NCT], f32)
        nc.scalar.activation(out=dec0[:], in_=ew[:], func=AF.Exp, scale=-1.0)  # e^-e^w
        decay = consts.tile([P, NCT], f32)
        nc.vector.tensor_scalar(out=decay[:], in0=dec0[:], scalar1=htf[:, 0:1],
                                scalar2=None, op0=ALU.mult)
        eu = consts.tile([P, NCT], f32)
        nc.scalar.activation(out=eu[:], in_=ut[:], func=AF.Exp)
        c1 = consts.tile([P, NCT], f32)      # e^u - 1
        nc.vector.tensor_scalar(out=c1[:], in0=eu[:], scalar1=1.0, scalar2=None,
                                op0=ALU.subtract)
        lnscale = consts.tile([P, 1], f32)   # ln(0.8 + 0.2*tf)
        nc.scalar.activation(out=lnscale[:], in_=scale_b[:], func=AF.Ln)

        # DRAM views: (tc tp) (j cc) -> tp tc j cc
        def slab_src(ap, j):
            return ap.rearrange("(tc tp) (j cc) -> tp tc j cc", tp=P, cc=P)[:, :, j, :]

        CH = min(512, T_)  # psum chunk width (one bank)
        NCH = T_ // CH    # chunks per ctile
        BPC = CH // P     # 128-blocks per chunk

        prev_dbc = None
        for j in range(NCT):
            kslab = slabs.tile([P, NT, P], f32, tag="kslab")
            vslab = slabs.tile([P, NT, P], f32, tag="vslab")
            rslab = slabs.tile([P, NT, P], f32, tag="rslab")
            nc.sync.dma_start(out=kslab[:], in_=slab_src(k, j))
            nc.sync.dma_start(out=vslab[:], in_=slab_src(v, j))
            nc.sync.dma_start(out=rslab[:], in_=slab_src(r, j))

            # ekvk holds ekv/ek interleaved along free dim: [ekv_t, ek_t, ...]
            # so ONE dual-scan instruction advances both recurrences.
            ekvk = cbuf.tile([P, 2 * T_], f32, tag="ekvk")
            erT = cbuf.tile([P, T_], f32, tag="erT", bufs=1)
            ek_v = ekvk[:, 1::2]
            ekv_v = ekvk[:, 0::2]
            for q in range(NCH):
                sl = slice(q * CH, (q + 1) * CH)
                isl = slice(2 * q * CH + 1, 2 * (q + 1) * CH, 2)
                vsl = slice(2 * q * CH, 2 * (q + 1) * CH, 2)
                pk = psum.tile([P, CH], f32, tag="pk")
                pv = psum.tile([P, CH], f32, tag="pv")
                pr = psum.tile([P, CH], f32, tag="pr")
                for s in range(BPC):
                    tcb = q * BPC + s
                    bs = slice(s * P, (s + 1) * P)
                    nc.tensor.transpose(pk[:, bs], kslab[:, tcb, :], ident[:])
                    nc.tensor.transpose(pv[:, bs], vslab[:, tcb, :], ident[:])
                    nc.tensor.transpose(pr[:, bs], rslab[:, tcb, :], ident[:])
                nc.scalar.activation(out=ekvk[:, isl], in_=pk[:], func=AF.Exp)
                nc.scalar.activation(out=erT[:, sl], in_=pr[:], func=AF.Exp,
                                     scale=-1.0)
                nc.vector.tensor_tensor(ekvk[:, vsl], ekvk[:, isl], pv[:],
                                        ALU.mult)

            # Stock scans on the interleaved views. (A custom dual-stream
            # 1-elem/cycle scan op was 2x faster in isolation but its
            # cross-stage flop feedback is input-stall-sensitive and corrupts
            # sparsely under real engine concurrency — reverted.)
            djb = decay[:, j:j + 1].broadcast_to((P, T_))
            nfdf = scano.tile([P, 2 * T_], f32, tag="nfdf", bufs=2)
            nbdb = scano.tile([P, 2 * T_], f32, tag="nbdb", bufs=2)
            nc.vector.tensor_tensor_scan(out=nfdf[:, 0::2], data0=djb,
                                         data1=ekvk[:, 0::2], initial=0.0,
                                         op0=ALU.mult, op1=ALU.add)
            nc.vector.tensor_tensor_scan(out=nfdf[:, 1::2], data0=djb,
                                         data1=ekvk[:, 1::2], initial=0.0,
                                         op0=ALU.mult, op1=ALU.add)
            nc.vector.tensor_tensor_scan(out=nbdb[:, 2 * T_ - 2::-2],
                                         data0=djb,
                                         data1=ekvk[:, 2 * T_ - 2::-2],
                                         initial=0.0,
                                         op0=ALU.mult, op1=ALU.add)
            nc.vector.tensor_tensor_scan(out=nbdb[:, 2 * T_ - 1::-2],
                                         data0=djb,
                                         data1=ekvk[:, 2 * T_ - 1::-2],
                                         initial=0.0,
                                         op0=ALU.mult, op1=ALU.add)

            cj = c1[:, j:j + 1]
            # num = nf + nb + c1*ekv ; den = df + db + c1*ek
            # out = num * exp(-(ln(den) + ln(1+e^-r)) + ln(scale_b))
            lnden = scano.tile([P, T_], f32, tag="lnden")
            o1 = scano.tile([P, T_], f32, tag="o1", bufs=2)
            NQ = 2
            H = T_ // NQ
            for h in range(NQ):
                fs = slice(2 * h * H, 2 * (h + 1) * H)     # interleaved slice
                es = slice(2 * h * H, 2 * (h + 1) * H, 2)  # ekv/nf slice
                os_ = slice(2 * h * H + 1, 2 * (h + 1) * H, 2)  # ek/df slice
                hs = slice(h * H, (h + 1) * H)
                # one stt covers both c1*ekv+nb and c1*ek+db (interleaved)
                nc.vector.scalar_tensor_tensor(out=nbdb[:, fs], in0=ekvk[:, fs],
                                               scalar=cj, in1=nbdb[:, fs],
                                               op0=ALU.mult, op1=ALU.add)
                # one add covers both num and den (interleaved)
                nc.vector.tensor_tensor(nfdf[:, fs], nfdf[:, fs], nbdb[:, fs],
                                        ALU.add)
                # l1p = ln(1 + e^-r) via Ln's fused bias; reuse erT in place
                nc.scalar.activation(out=erT[:, hs], in_=erT[:, hs], func=AF.Ln,
                                     bias=1.0)
                nc.scalar.activation(out=lnden[:, hs], in_=nfdf[:, os_],
                                     func=AF.Ln)
                nc.vector.tensor_tensor(lnden[:, hs], lnden[:, hs], erT[:, hs],
                                        ALU.add)
                nc.scalar.activation(out=lnden[:, hs], in_=lnden[:, hs],
                                     func=AF.Exp, scale=-1.0,
                                     bias=lnscale[:, 0:1])
                nc.vector.tensor_tensor(o1[:, hs], nfdf[:, es], lnden[:, hs],
                                        ALU.mult)

            oslab = slabs.tile([P, NT, P], f32, tag="oslab")
            for q in range(NCH):
                po = psum.tile([P, CH], f32, tag="po")
                for s in range(BPC):
                    bs = slice(s * P, (s + 1) * P)
                    tcb = q * BPC + s
                    nc.tensor.transpose(po[:, bs], o1[:, tcb * P:(tcb + 1) * P],
                                        ident[:])
                nc.scalar.activation(out=oslab[:, q * BPC:(q + 1) * BPC, :],
                                     in_=po[:], func=AF.Copy)
            nc.sync.dma_start(out=slab_src(out, j), in_=oslab[:])


def build_module(T_=T, C_=C, TD_=TD):
    nc = bacc.Bacc("TRN2", target_bir_lowering=False, debug=False)
    r = nc.dram_tensor("r", [T_, C_], f32, kind="ExternalInput").ap()
    k = nc.dram_tensor("k", [T_, C_], f32, kind="ExternalInput").ap()
    v = nc.dram_tensor("v", [T_, C_], f32, kind="ExternalInput").ap()
    w = nc.dram_tensor("w", [C_], f32, kind="ExternalInput").ap()
    u = nc.dram_tensor("u", [C_], f32, kind="ExternalInput").ap()
    te = nc.dram_tensor("time_emb", [TD_], f32, kind="ExternalInput").ap()
    out = nc.dram_tensor("out", [T_, C_], f32, kind="ExternalOutput").ap()
    with tile.TileContext(nc) as tc:
        _body(tc, out, r, k, v, w, u, te, T_, C_, TD_)
    nc.compile()
    return nc


_nc_cache = None


def run_full(r, k, v, w, u, time_emb, trace=False, **spmd_kwargs):
    """Run on 8 cores; returns (output [B,T,C], BassKernelResults)."""
    global _nc_cache
    if _nc_cache is None:
        _nc_cache = build_module()
    nc = _nc_cache
    r = np.asarray(r, dtype=np.float32)
    k = np.asarray(k, dtype=np.float32)
    v = np.asarray(v, dtype=np.float32)
    w = np.asarray(w, dtype=np.float32)
    u = np.asarray(u, dtype=np.float32)
    time_emb = np.asarray(time_emb, dtype=np.float32)
    in_maps = [
        {
            "r": np.ascontiguousarray(r[b]),
            "k": np.ascontiguousarray(k[b]),
            "v": np.ascontiguousarray(v[b]),
            "w": np.ascontiguousarray(w),
            "u": np.ascontiguousarray(u),
            "time_emb": np.ascontiguousarray(time_emb[b]),
        }
        for b in range(B)
    ]
    res = run_bass_kernel_spmd(nc, in_maps, core_ids=list(range(N_CORES)),
                               trace=trace, **spmd_kwargs)
    out = np.stack([res.results[b]["out"] for b in range(B)], axis=0)
    return out, res


def kernel(r, k, v, w, u, time_emb, **extra):
    out, _ = run_full(r, k, v, w, u, time_emb)
    return out

